# revision 3
# baseline (speedup 1.0000x reference)
"""Distributed contrastive-loss kernel for one TRN2 chip (8 NeuronCores).

loss = mean_i( logsumexp_j(l_ij) - l_{i,t_i} ),  l = (a_hat @ c_hat.T) / tau

Sharding: anchors data-parallel (2048 rows/core); each core normalizes its
OWN 2048-row candidate chunk, PE-transposes it to a k_sub-major fp8 layout,
and the transposed chunks are AllGathered on-chip (two j-halves, pipelined)
so every core reads the full candidate matrix with plain DMA loads.

Per-core pipeline:
  - Matmul: fp8e4 DoubleRow (K=256 as 128 partitions x 2 k-subtiles, one
    pass, 2 MACs/cycle). Operands are [p, ko, free] APs with ko stride
    = 2048 bytes (the tile_matmul-proven legal form). Anchors carry
    ra_i/tau so PSUM = 16*l (fp8 scale ranges: a*ra/tau ~ N(0,0.9),
    c_hat*16 ~ N(0,1)).
  - Exp is split across TWO consumers with SEPARATE 2-buf PSUM pools so
    their bank rotations never couple (a single shared pool loses ~2.5us
    per consumer switch): ScalarE does exact exp spans (scale=1/16,
    accum_out row-sums); DVE does 16-bit Schraudolph spans (f32->i16
    affine convert, bitcast bf16 ~= exp(l), 4x-mode accumulate).
    Ratio ~156:100 via Bresenham.
  - Target logits (host-gathered candidate rows) via bf16 dot/norm tasks
    dripped between span blocks.

The logits are bounded (|l| <= 14.3), so exp needs no max-subtraction.
"""

from collections import deque

import numpy as np

import concourse.bass as bass
import concourse.mybir as mybir
from concourse import bacc, tile, masks
from concourse.bass_utils import run_bass_kernel_spmd

F32 = mybir.dt.float32
BF16 = mybir.dt.bfloat16
F8 = mybir.dt.float8e4
I16 = mybir.dt.int16
ALU = mybir.AluOpType
ACTF = mybir.ActivationFunctionType

N_CORES = 8
N_FULL = 16384
M_FULL = 16384
D = 256
TAU = 0.07

SCH_S16 = 128.0 / np.log(2.0) / 16.0   # i16 = round(S*pm + B), pm = 16*l
SCH_B16 = 16248.64                     # calibrated: E[approx/exact] = 1
ND_SPANS = 100                         # of 256 spans, how many go to DVE


def _emit_rsqrt(nc, pool, x_ap, w, seed, iters=4):
    """Newton rsqrt on DVE: y' = y*(1.5 - 0.5*x*y^2), const seed."""
    y0 = pool.tile([128, w], F32, tag="nwt_y0")
    nc.vector.memset(y0[:], seed)
    y = y0[:]
    for _ in range(iters):
        t = pool.tile([128, w], F32, tag="nwt_t")
        nc.vector.tensor_mul(t[:], y, y)
        t2 = pool.tile([128, w], F32, tag="nwt_t2")
        nc.vector.scalar_tensor_tensor(t2[:], t[:], -0.5, x_ap,
                                       op0=ALU.mult, op1=ALU.mult)
        y2 = pool.tile([128, w], F32, tag="nwt_y2")
        nc.vector.scalar_tensor_tensor(y2[:], t2[:], 1.5, y,
                                       op0=ALU.add, op1=ALU.mult)
        y = y2[:]
    return y


def build_graph(NL=N_FULL // N_CORES, M=M_FULL, num_devices=N_CORES):
    NT = NL // 128         # anchor tiles per core (16)
    MC = M // N_CORES      # candidate chunk rows per core (2048)
    CT = MC // 128         # chunk tiles (16)
    MG = N_CORES           # candidate column groups = ranks
    SPW = 1024             # exp span width (2 PSUM banks)
    SP = MC // SPW         # spans per (t, g) = 2, also j-halves count
    JH = SPW               # j-half width in chunk rows

    nc = bacc.Bacc("TRN2", target_bir_lowering=False, debug=False,
                   num_devices=num_devices)

    anch = nc.dram_tensor("anch", [NL, D], F32, kind="ExternalInput")
    cchunk = nc.dram_tensor("cchunk", [MC, D], F32, kind="ExternalInput")
    tcand = nc.dram_tensor("tcand", [NL, D], F32, kind="ExternalInput")
    nll_out = nc.dram_tensor("nll", [128, NT], F32, kind="ExternalOutput")

    # transposed fp8 scratch, k_sub-major rows: d = ko*128 + ki
    scr_loc = [nc.dram_tensor(f"scr_loc{h}", [D, JH], F8, kind="Internal")
               for h in range(SP)]
    scr_all = [nc.dram_tensor(f"scr_all{h}", [MG * D, JH], F8, kind="Internal")
               for h in range(SP)]

    with tile.TileContext(nc) as tc:
        with (
            tc.tile_pool(name="persist", bufs=1) as persist,
            tc.tile_pool(name="work", bufs=2) as work,
            tc.tile_pool(name="small", bufs=2) as small,
            tc.tile_pool(name="nwt", bufs=2) as nwt,
            tc.tile_pool(name="psum_s", bufs=2, space="PSUM") as psum_s,
            tc.tile_pool(name="psum_d", bufs=2, space="PSUM") as psum_d,
        ):
            ident = persist.tile([128, 128], BF16, tag="ident")
            craw = persist.tile([128, CT * D], BF16, tag="craw")
            cns = persist.tile([128, CT], F32, tag="cns")
            cbf = persist.tile([128, CT * D], BF16, tag="cbf")
            ctT = persist.tile([128, 2 * MC], BF16, tag="ctT")  # [ki,(ko,j)]
            ct8s = [persist.tile([128, 2 * MC], F8, tag=f"ct8_{g}",
                                 name=f"ct8_{g}") for g in range(MG)]
            abf = persist.tile([128, NT * D], BF16, tag="abf")
            anormsq = persist.tile([128, NT], F32, tag="anormsq")
            ra_tau = persist.tile([128, NT], F32, tag="ra_tau")
            afs = persist.tile([128, NT * D], BF16, tag="afs")
            atT = persist.tile([128, 2 * NL], BF16, tag="atT")  # [ki,(ko,m)]
            at8 = persist.tile([128, 2 * NL], F8, tag="at8")
            tcb = persist.tile([128, NT * D], BF16, tag="tcb")
            tnormsq = persist.tile([128, NT], F32, tag="tnormsq")
            tdot = persist.tile([128, NT], F32, tag="tdot")
            ltgt = persist.tile([128, NT], F32, tag="ltgt")
            NSP = SP * NT * MG      # 256 spans total
            separts = persist.tile([128, NSP], F32, tag="separts")
            sumexp = persist.tile([128, NT], F32, tag="sumexp")
            sumexp2 = persist.tile([128, NT], F32, tag="sumexp2")
            lse = persist.tile([128, NT], F32, tag="lse")
            nll_sb = persist.tile([128, NT], F32, tag="nll_sb")

            masks.make_identity(nc, ident[:])

            # ---------------- head: issue input loads (cast f32->bf16) ----
            def cast_load(dst, src, rows0, ntiles):
                nc.gpsimd.dma_start(
                    dst.rearrange("p (j d) -> p j d", d=D),
                    src[rows0:rows0 + ntiles * 128, :]
                    .rearrange("(j p) d -> p j d", p=128))

            def norms_stt(src_bf, out_ns, t0, t1, tagp):
                for t in range(t0, t1):
                    sl = src_bf[:, t * D:(t + 1) * D]
                    tr = small.tile([128, D], BF16, tag="trashb",
                                    name=f"tr{tagp}{t}")
                    nc.vector.scalar_tensor_tensor(
                        tr[:], sl, 0.0, sl, op0=ALU.bypass, op1=ALU.mult,
                        accum_out=out_ns[:, t:t + 1])

            def transpose_pack(src_bf, dst_sb, ko, t0, nt):
                """Transpose nt tiles (d-half ko) of src_bf into
                dst_sb[:, ko*NL.. + t0*128 ..] via one PSUM pack."""
                ptr = psum_s.tile([128, nt * 128], BF16, tag="pm",
                                  name=f"ptr_{dst_sb.tensor.name}_{ko}_{t0}")
                for i in range(nt):
                    nc.tensor.transpose(
                        ptr[:, i * 128:(i + 1) * 128],
                        src_bf[:, (t0 + i) * D + ko * 128:
                               (t0 + i) * D + ko * 128 + 128],
                        ident[:])
                w = dst_sb.tensor.shape[1] // 2
                nc.vector.tensor_copy(
                    dst_sb[:, ko * w + t0 * 128: ko * w + (t0 + nt) * 128],
                    ptr[:])

            # C chunk: load halves, prep each j-half, SWDGE, gather
            cast_load(craw[:, :8 * D], cchunk, 0, 8)
            cast_load(craw[:, 8 * D:], cchunk, 8 * 128, 8)
            cast_load(abf[:], anch, 0, NT)       # A load in parallel
            nc.gpsimd.dma_start(                  # tc load (consumed late)
                tcb[:].rearrange("p (j d) -> p j d", d=D),
                tcand[:, :].rearrange("(j p) d -> p j d", p=128))

            rc16s = []
            for th in range(2):
                t0, t1 = th * 8, (th + 1) * 8
                norms_stt(craw, cns, t0, t1, "c")
                rc = _emit_rsqrt(nc, nwt, cns[:, t0:t1], 8, seed=D ** -0.5)
                rc16 = small.tile([128, 8], F32, tag="rc16", name=f"rc16_{th}")
                nc.vector.tensor_scalar_mul(rc16[:], rc, 16.0)
                rc16s.append(rc16)
                for j in range(t0, t1):
                    nc.vector.tensor_scalar(
                        cbf[:, j * D:(j + 1) * D], craw[:, j * D:(j + 1) * D],
                        rc16[:, j - t0:j - t0 + 1], None, op0=ALU.mult)
                for ko in range(2):
                    transpose_pack(cbf, ctT, ko, t0, 8)
                # SWDGE cast write bf16 -> fp8 for this j-half
                nc.gpsimd.dma_start(
                    scr_loc[th][:, :].rearrange("(ko p) j -> p ko j", p=128),
                    ctT[:].rearrange("p (ko j) -> p ko j", ko=2)
                    [:, :, th * JH:(th + 1) * JH])
                nc.gpsimd.collective_compute(
                    "AllGather", ALU.bypass,
                    replica_groups=[list(range(num_devices))],
                    ins=[scr_loc[th][:, :]],
                    outs=[scr_all[th][:, :]],
                )

            # ---- A prep (overlaps the gathers) ----
            norms_stt(abf, anormsq, 0, NT, "a")
            ra = _emit_rsqrt(nc, nwt, anormsq[:], NT, seed=D ** -0.5)
            nc.vector.tensor_scalar_mul(ra_tau[:], ra, 1.0 / TAU)
            for t in range(NT):
                nc.vector.tensor_scalar(
                    afs[:, t * D:(t + 1) * D], abf[:, t * D:(t + 1) * D],
                    ra_tau[:, t:t + 1], None, op0=ALU.mult)
            for ko in range(2):
                for th in range(2):
                    transpose_pack(afs, atT, ko, th * 8, 8)
            nc.vector.tensor_copy(at8[:], atT[:])   # bf16 -> fp8

            # ---- ct8 group loads (h0 now; h1 after gather completes) ----
            def load_ct8_half(g, h):
                nc.sync.dma_start(
                    ct8s[g][:].rearrange("p (ko j) -> p ko j", ko=2)
                    [:, :, h * JH:(h + 1) * JH],
                    scr_all[h][g * D:(g + 1) * D, :]
                    .rearrange("(ko p) j -> p ko j", p=128))

            for g in range(MG):
                load_ct8_half(g, 0)

            # ---- tc tasks dripped into the main loop ----
            tasks = deque()

            def tc_norm_task(t0):
                for t in range(t0, t0 + 4):
                    tsl = tcb[:, t * D:(t + 1) * D]
                    tr = small.tile([128, D], BF16, tag="trashb",
                                    name=f"trt{t}")
                    nc.vector.scalar_tensor_tensor(
                        tr[:], tsl, 0.0, tsl, op0=ALU.bypass, op1=ALU.mult,
                        accum_out=tnormsq[:, t:t + 1])

            def tc_dot_task(t0):
                for t in range(t0, t0 + 4):
                    tsl = tcb[:, t * D:(t + 1) * D]
                    tr2 = small.tile([128, D], BF16, tag="trashb",
                                     name=f"trd{t}")
                    nc.vector.scalar_tensor_tensor(
                        tr2[:], abf[:, t * D:(t + 1) * D], 0.0, tsl,
                        op0=ALU.bypass, op1=ALU.mult,
                        accum_out=tdot[:, t:t + 1])

            def tc_finish():
                rtc = _emit_rsqrt(nc, nwt, tnormsq[:], NT, seed=D ** -0.5)
                tmp2 = small.tile([128, NT], F32, tag="ltg2")
                nc.vector.tensor_mul(tmp2[:], tdot[:], ra_tau[:])
                nc.vector.tensor_mul(ltgt[:], tmp2[:], rtc)

            for t0 in range(0, NT, 4):
                tasks.append(lambda t0=t0: tc_norm_task(t0))
            for t0 in range(0, NT, 4):
                tasks.append(lambda t0=t0: tc_dot_task(t0))
            tasks.append(tc_finish)

            # ---- main loop: hg outer, then t, then g ----
            span_idx = 0
            d_emitted = 0
            for hg in range(SP):
                if hg > 0:
                    for g in range(MG):
                        load_ct8_half(g, hg)
                for t in range(NT):
                    lhsT = at8[:].rearrange("p (ko m) -> p ko m", ko=2)[
                        :, :, t * 128:(t + 1) * 128]
                    for g in range(MG):
                        if tasks:
                            tasks.popleft()()
                        k = (hg * NT + t) * MG + g
                        want_d = ((span_idx + 1) * ND_SPANS) // NSP
                        is_d = want_d > d_emitted
                        if is_d:
                            d_emitted += 1
                        pool = psum_d if is_d else psum_s
                        pm = pool.tile([128, SPW], F32, tag="pm",
                                       name=f"pm{k}")
                        rhs = ct8s[g][:].rearrange("p (ko j) -> p ko j", ko=2)
                        for sc in range(SPW // 512):
                            nc.tensor.matmul(
                                pm[:, sc * 512:(sc + 1) * 512],
                                lhsT=lhsT,
                                rhs=rhs[:, :, hg * SPW + sc * 512:
                                        hg * SPW + (sc + 1) * 512],
                                start=True, stop=True,
                                perf_mode=mybir.MatmulPerfMode.DoubleRow)
                        if is_d:
                            ei = work.tile([128, SPW], I16, tag="ei",
                                           name=f"ei{k}")
                            nc.vector.tensor_scalar(
                                ei[:], pm[:], SCH_S16, SCH_B16,
                                op0=ALU.mult, op1=ALU.add)
                            etr = work.tile([128, SPW], BF16, tag="etrd",
                                            name=f"etrd{k}")
                            nc.vector.tensor_scalar(
                                etr[:], ei[:].bitcast(BF16), 1.0, None,
                                op0=ALU.mult, op1=ALU.add,
                                accum_out=separts[:, k:k + 1])
                        else:
                            etr = work.tile([128, SPW], BF16, tag="etrs",
                                            name=f"etrs{k}")
                            nc.scalar.activation(
                                etr[:], pm[:], ACTF.Exp, scale=1.0 / 16.0,
                                accum_out=separts[:, k:k + 1])
                        span_idx += 1

            while tasks:
                tasks.popleft()()

            # ---- finalize ----
            nc.vector.reduce_sum(
                sumexp[:],
                separts[:, :NT * MG].rearrange("p (t g) -> p t g", t=NT),
                axis=mybir.AxisListType.X)
            nc.vector.reduce_sum(
                sumexp2[:],
                separts[:, NT * MG:].rearrange("p (t g) -> p t g", t=NT),
                axis=mybir.AxisListType.X)
            nc.vector.tensor_add(sumexp[:], sumexp[:], sumexp2[:])
            nc.scalar.activation(lse[:], sumexp[:], ACTF.Ln)
            nc.vector.tensor_sub(nll_sb[:], lse[:], ltgt[:])
            nc.gpsimd.dma_start(nll_out[:, :], nll_sb[:])

    nc.compile()
    return nc


_CACHE = {}


def _compiled():
    if "nc" not in _CACHE:
        _CACHE["nc"] = build_graph()
    return _CACHE["nc"]


def make_in_maps(anchors, candidates, targets):
    anchors = np.ascontiguousarray(np.asarray(anchors, dtype=np.float32))
    candidates = np.ascontiguousarray(np.asarray(candidates, dtype=np.float32))
    targets = np.asarray(targets, dtype=np.int32)
    tc_full = candidates[targets]          # [N, D] host gather of target rows
    nl = anchors.shape[0] // N_CORES
    mc = candidates.shape[0] // N_CORES
    in_maps = []
    for c in range(N_CORES):
        sl = slice(c * nl, (c + 1) * nl)
        in_maps.append({
            "anch": np.ascontiguousarray(anchors[sl]),
            "cchunk": np.ascontiguousarray(candidates[c * mc:(c + 1) * mc]),
            "tcand": np.ascontiguousarray(tc_full[sl]),
        })
    return in_maps


def kernel(anchors, candidates, targets):
    nc = _compiled()
    in_maps = make_in_maps(anchors, candidates, targets)
    res = run_bass_kernel_spmd(nc, in_maps, core_ids=list(range(N_CORES)))
    nll = np.stack([np.asarray(r["nll"], dtype=np.float64)
                    for r in res.results])
    return np.float32(nll.mean())


# revision 4
# speedup vs baseline: 1.1076x; 1.1076x over previous
"""Distributed contrastive-loss kernel for one TRN2 chip (8 NeuronCores).

loss = mean_i( logsumexp_j(l_ij) - l_{i,t_i} ),  l = (a_hat @ c_hat.T) / tau

Sharding: anchors data-parallel (2048 rows/core); each core normalizes its
OWN 2048-row candidate chunk, PE-transposes it to a k_sub-major fp8 layout
(k = ko*128 + ki), casts bf16->fp8 in the SWDGE write, and the transposed
chunks are AllGathered on-chip so every core reads the full candidate
matrix with plain DMA loads (tile_matmul-proven [p, ko, j] AP form).

Per-core pipeline:
  - Matmul: fp8e4 DoubleRow, K=256 in one pass (2 MACs/cycle). Anchors
    carry ra_i/tau so PSUM = 16*l (fp8 ranges: a*ra/tau ~ N(0,0.9),
    c_hat*16 ~ N(0,1)); logit quantization error ~0.05 washes out in the
    16384-term softmax sums and the 16384-row mean.
  - The exp stream (33.5M elems/core) is split across TWO consumers with
    SEPARATE PSUM pools so their bank rotations never couple:
      ScalarE: exact exp spans (scale=1/16, fused accum row-sums);
      DVE: 16-bit Schraudolph: f32->i16 affine convert (the i16 IS the
        bf16 bit pattern of ~exp(l)), then ONE batched 1x CACHE_REDUCE
        per 4 blocks over the bitcast-bf16 buffer (accum ops have no
        packed mode, so batching amortizes the fixed costs).
    Per 2048-col block: mostly [S:1536 | D:512], every ~5th block
    [S:1024 | D:1024] to balance engine busy times.
  - Target logits (host-gathered candidate rows) via bf16 dot/norm tasks
    dripped between blocks.

The logits are bounded (|l| <= 14.3), so exp needs no max-subtraction.
"""

from collections import deque

import numpy as np

import concourse.bass as bass
import concourse.mybir as mybir
from concourse import bacc, tile, masks
from concourse.bass_utils import run_bass_kernel_spmd

F32 = mybir.dt.float32
BF16 = mybir.dt.bfloat16
F8 = mybir.dt.float8e4
I16 = mybir.dt.int16
ALU = mybir.AluOpType
ACTF = mybir.ActivationFunctionType

N_CORES = 8
N_FULL = 16384
M_FULL = 16384
D = 256
TAU = 0.07

SCH_S16 = 128.0 / np.log(2.0) / 16.0   # i16 = round(S*pm + B), pm = 16*l
SCH_B16 = 16248.64                     # calibrated: E[approx/exact] = 1
NV_BLOCKS = 25                         # blocks with the wider DVE share


def _emit_rsqrt(nc, pool, x_ap, w, seed, iters=3):
    """Newton rsqrt on DVE: y' = y*(1.5 - 0.5*x*y^2), const seed.
    Inputs concentrate near D (chi^2), so 3 iters reach ~1e-4 worst-case."""
    y0 = pool.tile([128, w], F32, tag="nwt_y0")
    nc.vector.memset(y0[:], seed)
    y = y0[:]
    for _ in range(iters):
        t = pool.tile([128, w], F32, tag="nwt_t")
        nc.vector.tensor_mul(t[:], y, y)
        t2 = pool.tile([128, w], F32, tag="nwt_t2")
        nc.vector.scalar_tensor_tensor(t2[:], t[:], -0.5, x_ap,
                                       op0=ALU.mult, op1=ALU.mult)
        y2 = pool.tile([128, w], F32, tag="nwt_y2")
        nc.vector.scalar_tensor_tensor(y2[:], t2[:], 1.5, y,
                                       op0=ALU.add, op1=ALU.mult)
        y = y2[:]
    return y


def build_graph(NL=N_FULL // N_CORES, M=M_FULL, num_devices=N_CORES):
    NT = NL // 128         # anchor tiles per core (16)
    MC = M // N_CORES      # candidate chunk rows per core (2048)
    CT = MC // 128         # chunk tiles (16)
    MG = N_CORES           # candidate column groups = ranks

    nc = bacc.Bacc("TRN2", target_bir_lowering=False, debug=False,
                   num_devices=num_devices)

    anch = nc.dram_tensor("anch", [NL, D], F32, kind="ExternalInput")
    cchunk = nc.dram_tensor("cchunk", [MC, D], F32, kind="ExternalInput")
    tcand = nc.dram_tensor("tcand", [NL, D], F32, kind="ExternalInput")
    nll_out = nc.dram_tensor("nll", [128, NT], F32, kind="ExternalOutput")

    # transposed fp8 scratch, k_sub-major rows: d = ko*128 + ki
    scr_loc = nc.dram_tensor("scr_loc", [D, MC], F8, kind="Internal")
    scr_all = nc.dram_tensor("scr_all", [MG * D, MC], F8, kind="Internal")

    RPT = MG + 2           # separts partials per t: 8 S + up to 2 D flushes

    with tile.TileContext(nc) as tc:
        with (
            tc.tile_pool(name="persist", bufs=1) as persist,
            tc.tile_pool(name="work", bufs=2) as work,
            tc.tile_pool(name="small", bufs=2) as small,
            tc.tile_pool(name="nwt", bufs=2) as nwt,
            tc.tile_pool(name="psum_s", bufs=2, space="PSUM") as psum_s,
            tc.tile_pool(name="psum_d", bufs=2, space="PSUM") as psum_d,
        ):
            ident = persist.tile([128, 128], BF16, tag="ident")
            craw = persist.tile([128, CT * D], BF16, tag="craw")
            cns = persist.tile([128, CT], F32, tag="cns")
            cbf = persist.tile([128, CT * D], BF16, tag="cbf")
            ctT = persist.tile([128, 2 * MC], BF16, tag="ctT")  # [ki,(ko,j)]
            ct8s = [persist.tile([128, 2 * MC], F8, tag=f"ct8_{g}",
                                 name=f"ct8_{g}") for g in range(MG)]
            abf = persist.tile([128, NT * D], BF16, tag="abf")
            anormsq = persist.tile([128, NT], F32, tag="anormsq")
            ra_tau = persist.tile([128, NT], F32, tag="ra_tau")
            afs = persist.tile([128, NT * D], BF16, tag="afs")
            atT = persist.tile([128, 2 * NL], BF16, tag="atT")  # [ki,(ko,m)]
            at8 = persist.tile([128, 2 * NL], F8, tag="at8")
            tcb = persist.tile([128, NT * D], BF16, tag="tcb")
            tnormsq = persist.tile([128, NT], F32, tag="tnormsq")
            tdot = persist.tile([128, NT], F32, tag="tdot")
            ltgt = persist.tile([128, NT], F32, tag="ltgt")
            separts = persist.tile([128, NT * RPT], F32, tag="separts")
            sumexp = persist.tile([128, NT], F32, tag="sumexp")
            lse = persist.tile([128, NT], F32, tag="lse")
            nll_sb = persist.tile([128, NT], F32, tag="nll_sb")

            masks.make_identity(nc, ident[:])
            nc.vector.memset(separts[:], 0.0)

            def cast_load(dst, src, rows0, ntiles):
                nc.gpsimd.dma_start(
                    dst.rearrange("p (j d) -> p j d", d=D),
                    src[rows0:rows0 + ntiles * 128, :]
                    .rearrange("(j p) d -> p j d", p=128))

            def norms_stt(src_bf, out_ns, t0, t1, tagp):
                for t in range(t0, t1):
                    sl = src_bf[:, t * D:(t + 1) * D]
                    tr = small.tile([128, D], BF16, tag="trashb",
                                    name=f"tr{tagp}{t}")
                    nc.vector.scalar_tensor_tensor(
                        tr[:], sl, 0.0, sl, op0=ALU.bypass, op1=ALU.mult,
                        accum_out=out_ns[:, t:t + 1])

            def transpose_pack(src_bf, dst_sb, ko, t0, nt):
                """Transpose nt tiles (d-half ko) of src_bf into
                dst_sb[:, ko*W + t0*128 ..] via one PSUM pack."""
                ptr = psum_s.tile([128, nt * 128], BF16, tag="pm",
                                  name=f"ptr_{dst_sb.tensor.name}_{ko}_{t0}")
                for i in range(nt):
                    nc.tensor.transpose(
                        ptr[:, i * 128:(i + 1) * 128],
                        src_bf[:, (t0 + i) * D + ko * 128:
                               (t0 + i) * D + ko * 128 + 128],
                        ident[:])
                w = dst_sb.tensor.shape[1] // 2
                nc.vector.tensor_copy(
                    dst_sb[:, ko * w + t0 * 128: ko * w + (t0 + nt) * 128],
                    ptr[:])

            # ---------------- head ----------------
            cast_load(craw[:, :8 * D], cchunk, 0, 8)
            cast_load(craw[:, 8 * D:], cchunk, 8 * 128, 8)
            cast_load(abf[:], anch, 0, NT)
            nc.gpsimd.dma_start(
                tcb[:].rearrange("p (j d) -> p j d", d=D),
                tcand[:, :].rearrange("(j p) d -> p j d", p=128))

            # C chunk prep
            norms_stt(craw, cns, 0, CT, "c")
            rc = _emit_rsqrt(nc, nwt, cns[:], CT, seed=D ** -0.5)
            rc16 = small.tile([128, CT], F32, tag="rc16")
            nc.vector.tensor_scalar_mul(rc16[:], rc, 16.0)
            for j in range(CT):
                nc.vector.tensor_scalar(
                    cbf[:, j * D:(j + 1) * D], craw[:, j * D:(j + 1) * D],
                    rc16[:, j:j + 1], None, op0=ALU.mult)
            for ko in range(2):
                for th in range(2):
                    transpose_pack(cbf, ctT, ko, th * 8, 8)
            nc.gpsimd.dma_start(
                scr_loc[:, :].rearrange("(ko p) j -> p ko j", p=128),
                ctT[:].rearrange("p (ko j) -> p ko j", ko=2))
            nc.gpsimd.collective_compute(
                "AllGather", ALU.bypass,
                replica_groups=[list(range(num_devices))],
                ins=[scr_loc[:, :]],
                outs=[scr_all[:, :]],
            )

            # A prep (overlaps the gather)
            norms_stt(abf, anormsq, 0, NT, "a")
            ra = _emit_rsqrt(nc, nwt, anormsq[:], NT, seed=D ** -0.5)
            nc.vector.tensor_scalar_mul(ra_tau[:], ra, 1.0 / TAU)
            for t in range(NT):
                nc.vector.tensor_scalar(
                    afs[:, t * D:(t + 1) * D], abf[:, t * D:(t + 1) * D],
                    ra_tau[:, t:t + 1], None, op0=ALU.mult)
            for ko in range(2):
                for th in range(2):
                    transpose_pack(afs, atT, ko, th * 8, 8)
            nc.vector.tensor_copy(at8[:], atT[:])   # bf16 -> fp8

            # ct8 group loads (after gather)
            for g in range(MG):
                nc.sync.dma_start(
                    ct8s[g][:].rearrange("p (ko j) -> p ko j", ko=2),
                    scr_all[g * D:(g + 1) * D, :]
                    .rearrange("(ko p) j -> p ko j", p=128))

            # ---- tc tasks dripped into the main loop ----
            tasks = deque()

            def tc_norm_task(t0):
                norms_stt(tcb, tnormsq, t0, t0 + 2, "t")

            def tc_dot_task(t0):
                for t in range(t0, t0 + 2):
                    tsl = tcb[:, t * D:(t + 1) * D]
                    tr2 = small.tile([128, D], BF16, tag="trashb",
                                     name=f"trd{t}")
                    nc.vector.scalar_tensor_tensor(
                        tr2[:], abf[:, t * D:(t + 1) * D], 0.0, tsl,
                        op0=ALU.bypass, op1=ALU.mult,
                        accum_out=tdot[:, t:t + 1])

            def tc_finish():
                rtc = _emit_rsqrt(nc, nwt, tnormsq[:], NT, seed=D ** -0.5)
                tmp2 = small.tile([128, NT], F32, tag="ltg2")
                nc.vector.tensor_mul(tmp2[:], tdot[:], ra_tau[:])
                nc.vector.tensor_mul(ltgt[:], tmp2[:], rtc)

            for t0 in range(0, NT, 2):
                tasks.append(lambda t0=t0: tc_norm_task(t0))
            for t0 in range(0, NT, 2):
                tasks.append(lambda t0=t0: tc_dot_task(t0))
            tasks.append(tc_finish)

            # ---- main loop: blocks (t, g) of 2048 cols ----
            NB = NT * MG
            b_idx = 0
            v_emitted = 0
            for t in range(NT):
                lhsT = at8[:].rearrange("p (ko m) -> p ko m", ko=2)[
                    :, :, t * 128:(t + 1) * 128]
                eib = work.tile([128, 8192], I16, tag="eib", name=f"eib{t}")
                ei_off = 0
                flushes = 0

                def flush_d(force=False):
                    nonlocal ei_off, flushes
                    if ei_off == 0:
                        return
                    k = t * RPT + MG + flushes
                    nc.vector.tensor_scalar(
                        work.tile([128, ei_off], BF16, tag="etrd",
                                  name=f"etrd{t}_{flushes}")[:],
                        eib[:, :ei_off].bitcast(BF16), 1.0, None,
                        op0=ALU.mult, op1=ALU.add,
                        accum_out=separts[:, k:k + 1])
                    flushes += 1
                    ei_off = 0

                for g in range(MG):
                    if tasks:
                        tasks.popleft()()
                    want_v = ((b_idx + 1) * NV_BLOCKS) // NB
                    is_v = want_v > v_emitted
                    if is_v:
                        v_emitted += 1
                    sw = 1024 if is_v else 1536      # scalar span width
                    rhs = ct8s[g][:].rearrange("p (ko j) -> p ko j", ko=2)

                    pm_s = psum_s.tile([128, sw], F32, tag="pm",
                                       name=f"pms{b_idx}")
                    for sc in range(sw // 512):
                        nc.tensor.matmul(
                            pm_s[:, sc * 512:(sc + 1) * 512],
                            lhsT=lhsT,
                            rhs=rhs[:, :, sc * 512:(sc + 1) * 512],
                            start=True, stop=True,
                            perf_mode=mybir.MatmulPerfMode.DoubleRow)
                    nd = (2048 - sw) // 512          # D sub-spans (1 or 2)
                    pm_ds = []
                    for q in range(nd):
                        col = sw + q * 512
                        pm_d = psum_d.tile([128, 512], F32, tag="pm",
                                           name=f"pmd{b_idx}_{q}")
                        nc.tensor.matmul(
                            pm_d[:],
                            lhsT=lhsT,
                            rhs=rhs[:, :, col:col + 512],
                            start=True, stop=True,
                            perf_mode=mybir.MatmulPerfMode.DoubleRow)
                        pm_ds.append(pm_d)

                    k = t * RPT + g
                    etr = work.tile([128, sw], BF16, tag="etrs",
                                    name=f"etrs{b_idx}")
                    nc.scalar.activation(
                        etr[:], pm_s[:], ACTF.Exp, scale=1.0 / 16.0,
                        accum_out=separts[:, k:k + 1])
                    for q in range(nd):
                        nc.vector.tensor_scalar(
                            eib[:, ei_off:ei_off + 512], pm_ds[q][:],
                            SCH_S16, SCH_B16, op0=ALU.mult, op1=ALU.add)
                        ei_off += 512
                    if g == 3 or g == MG - 1:
                        flush_d()
                    b_idx += 1

            while tasks:
                tasks.popleft()()

            # ---- finalize ----
            nc.vector.reduce_sum(
                sumexp[:],
                separts[:].rearrange("p (t r) -> p t r", t=NT),
                axis=mybir.AxisListType.X)
            nc.scalar.activation(lse[:], sumexp[:], ACTF.Ln)
            nc.vector.tensor_sub(nll_sb[:], lse[:], ltgt[:])
            nc.gpsimd.dma_start(nll_out[:, :], nll_sb[:])

    nc.compile()
    return nc


_CACHE = {}


def _compiled():
    if "nc" not in _CACHE:
        _CACHE["nc"] = build_graph()
    return _CACHE["nc"]


def make_in_maps(anchors, candidates, targets):
    anchors = np.ascontiguousarray(np.asarray(anchors, dtype=np.float32))
    candidates = np.ascontiguousarray(np.asarray(candidates, dtype=np.float32))
    targets = np.asarray(targets, dtype=np.int32)
    tc_full = candidates[targets]          # [N, D] host gather of target rows
    nl = anchors.shape[0] // N_CORES
    mc = candidates.shape[0] // N_CORES
    in_maps = []
    for c in range(N_CORES):
        sl = slice(c * nl, (c + 1) * nl)
        in_maps.append({
            "anch": np.ascontiguousarray(anchors[sl]),
            "cchunk": np.ascontiguousarray(candidates[c * mc:(c + 1) * mc]),
            "tcand": np.ascontiguousarray(tc_full[sl]),
        })
    return in_maps


def kernel(anchors, candidates, targets):
    nc = _compiled()
    in_maps = make_in_maps(anchors, candidates, targets)
    res = run_bass_kernel_spmd(nc, in_maps, core_ids=list(range(N_CORES)))
    nll = np.stack([np.asarray(r["nll"], dtype=np.float64)
                    for r in res.results])
    return np.float32(nll.mean())
